# revision 8
# baseline (speedup 1.0000x reference)
"""Trainium2 Bass kernel for nn_ExteriorDerivative (d of a 2-form via central FD).

Math: the reference's central FD collapses analytically to out = cos(x@W1) @ G
with G the (32, 35) fold of the FD gather/sign/scatter pipeline (built in fp64
on host from W1/W2 only); on-device cos(z) = 1 - 2*sin^2(z/2) (ACT Sin + a
square), the affine part folded into mm2 (-2G) plus per-output-row bias g1
added during the PSUM->SBUF output copies.

Layout (per core, B_CORE=32768): 4 batch subgroups stacked block-diagonally:
  xt fp16 [28, 8192] (cols 0:1024 ride in the critical 'wab' load with W1),
  z PSUM [128, 1024] per group, q fp16 [128, 1024], 8 groups of 1024 cols.
mm2 needs 4*35=140 output rows > 128 PSUM partitions, so it is split:
  A: first 128 rows (subgroups 0-2 full + 23 rows of subgroup 3) per group;
  B: last 12 rows of subgroup 3, densified across each TRIPLE of groups into
     one [96, 1024] PSUM tile (group 3p+k writes a 32-row stripe at partition
     32k - matmul PSUM writes must start at 0/32/64 - with the gB stationary
     zero-padded to 32 columns so stripes are fully initialized), copied once
     per triple with the g1B bias as a per-partition scalar.

Schedule (all knobs above; every value is a measured two-sided optimum):
  - sins on ACT, back-to-back; exactly 2 ring-tax A-copies ride in-window;
  - squares split DVE/Pool by column half ('s'), group 0 pure-DVE (a Pool
    share there starts the Pool drift chain that paces the tail);
  - A-copies alternate DVE/ACT; emission order is squares THEN deferred
    copies (a copy before the square head-of-line-blocks the in-order DVE);
  - mm1 runs two groups ahead of mm2 through a shared 3-tile PSUM ring
    (zp and opA alternate; 6 banks + 2 for the B tile = all 8);
  - late stores are unpaired so their transfers don't serialize the tail;
  - TRN2 constraints: GPSIMD and DMA cannot touch PSUM (every PSUM read is
    ACT or DVE); per-DMA latency is ~625 (HWDGE) + 650 (DGE) + 900 (sem).

TimelineSim = 21325 ns/core; HW-verified rel err 1.3e-3 (gate 2e-2).
"""
import numpy as np
from itertools import combinations

DIM = 7
EPS = 1e-4
NCORES = 8
B = 262144
B_CORE = B // NCORES          # 32768
SUB = 4
K_IN = SUB * DIM              # 28
M1 = SUB * 32                 # 128
COLS = B_CORE // SUB          # 8192
TILE_N = 512

# --- schedule knobs ---
GSCHED = [1024] * 8                       # group widths (sum == COLS)
XCHUNKS = [1024, 1024, 2048, 2048, 1024]  # chunk sizes for cols 1024.. (sum == COLS-1024)
SQ_ENG = list("dsssssss")                 # d=DVE p=Pool s=split DVE/Pool
CP_ENG = list("dadadada")                 # per-group A-copy engine: a=ACT d=DVE
CPB_ENG = "ddd"                           # B-copy engine per triple
STORE_PAIR = [(0, 1), (2, 3), (4,), (5,), (6,), (7,)]  # groups per A-store
ACT_DEFER = 2                             # groups to defer ACT copies
DVE_DEFER = 2
WBUFS = 6
POOL_COLS = 512                           # Pool's share of an 's' split square
OCH_BUFS = 5

# ---- static exterior-derivative index maps (mirrors reference.py) ----
_IDX3 = list(combinations(range(DIM), 3))
_POS2 = {t: i for i, t in enumerate(combinations(range(DIM), 2))}
_D2 = []
for _out, (i, j, k) in enumerate(_IDX3):
    for _p, (a, b, c) in enumerate([(i, j, k), (j, i, k), (k, i, j)]):
        bc = tuple(sorted((b, c)))
        s = (-1) ** _p * (1 if (b, c) == bc else -1)
        _D2.append((_out, _POS2[bc], a, s))


def _build_G(W1: np.ndarray, W2: np.ndarray) -> np.ndarray:
    """G[j, o] = sum_t SIGNS[t] * sin(EPS*W1[DCOORD[t], j])/EPS * W2[j, IN_POS[t]]."""
    W1d = W1.astype(np.float64)
    W2d = W2.astype(np.float64)
    G = np.zeros((32, 35), dtype=np.float64)
    for out_pos, in_pos, dcoord, sign in _D2:
        G[:, out_pos] += sign * (np.sin(EPS * W1d[dcoord, :]) / EPS) * W2d[:, in_pos]
    return G


_PROG = None


def _get_prog():
    global _PROG
    if _PROG is not None:
        return _PROG
    import concourse.bacc as bacc
    import concourse.bass as bass
    import concourse.tile as tile
    import concourse.mybir as mybir

    F32 = mybir.dt.float32
    F16 = mybir.dt.float16
    Sin = mybir.ActivationFunctionType.Sin
    Ident = mybir.ActivationFunctionType.Identity
    Square = mybir.ActivationFunctionType.Square
    Alu = mybir.AluOpType

    assert sum(GSCHED) == COLS and sum(XCHUNKS) == COLS - 1024

    nc = bacc.Bacc("TRN2", target_bir_lowering=False, debug=False, num_devices=NCORES)
    xt = nc.dram_tensor("xt", [K_IN, COLS], F16, kind="ExternalInput")
    wab = nc.dram_tensor("wab", [K_IN, 1152], F16, kind="ExternalInput")
    wgb = nc.dram_tensor("wgb", [128, 160], F16, kind="ExternalInput")
    g1ab = nc.dram_tensor("g1ab", [128, 2], F32, kind="ExternalInput")
    ot = nc.dram_tensor("ot", [128, COLS], F16, kind="ExternalOutput")
    otb = nc.dram_tensor("otb", [76, 3072], F16, kind="ExternalOutput")

    # group -> (start col, width); chunk boundaries (chunks start at col 1024:
    # the first 1024 x-cols ride in the wab load)
    XBASE = 1024
    gstart = np.cumsum([0] + GSCHED).tolist()
    cstart = np.cumsum([XBASE] + XCHUNKS).tolist()

    with tile.TileContext(nc) as tc:
        with (
            tc.tile_pool(name="singles", bufs=1) as singles,
            tc.tile_pool(name="xin", bufs=len(XCHUNKS)) as xpool,
            tc.tile_pool(name="och", bufs=OCH_BUFS) as opool,
            tc.tile_pool(name="work", bufs=WBUFS) as wpool,
            tc.tile_pool(name="ps", bufs=3, space=bass.MemorySpace.PSUM) as pspool,
            tc.tile_pool(name="psb", bufs=1, space=bass.MemorySpace.PSUM) as psbpool,
        ):
            was = singles.tile([K_IN, 1152], F16)
            nc.sync.dma_start(was[:], wab[:])
            wgs = singles.tile([128, 160], F16)
            nc.sync.dma_start(wgs[:], wgb[:])
            g1s = singles.tile([128, 2], F32)
            zbias = singles.tile([128, 1], F32)
            nc.vector.memset(zbias[:], 0.0)

            gas = wgs[:, 0:128]
            gbs = wgs[64:128, 128:160]
            w1s = was[0:28, 0:128]
            g1as = g1s[:, 0:1]
            g1bs = g1s[0:76, 1:2]

            xin_tiles = {}      # chunk idx -> (tile, c0, cn)
            btiles = {}         # triple idx -> psum tile
            next_chunk = 0

            def load_chunk(ci):
                c0, cn = cstart[ci], XCHUNKS[ci]
                t = xpool.tile([K_IN, cn], F16, tag=f"xin{ci}")
                q = nc.gpsimd if ci == 0 else nc.sync
                q.dma_start(t[:, :cn], xt[:, c0:c0 + cn])
                xin_tiles[ci] = (t, c0, cn)

            load_chunk(0)
            g1_loaded = False

            NG = len(GSCHED)
            copy_jobs = {}      # gi -> closure
            och_tiles = {}      # pair idx -> (tile, base col, width, ngroups left)
            pair_of = {}
            for pi, pr in enumerate(STORE_PAIR):
                for gg in pr:
                    pair_of[gg] = pi

            def emit_copy(gi):
                copy_jobs.pop(gi)()

            def emit_store(pi):
                tile_, base, width = och_tiles[pi]
                nc.sync.dma_start(ot[:, base:base + width], tile_[:, :width])

            pending_stores = {}  # pair -> n copies remaining

            for pr in STORE_PAIR:
                pending_stores[pair_of[pr[0]]] = len(pr)

            bparts = {}  # p -> och tile

            def emit_bcopy(p, r0, r1, width=1024, eng=None):
                """Copy btile[p] rows r0:r1 (cols :width) out, with g1 bias."""
                bt = btiles[p]
                if p not in bparts:
                    obt = opool.tile([76, 1024], F16, tag="ochB")
                    bparts[p] = obt
                ob = bparts[p]
                eng = eng or CPB_ENG[p]
                bias = g1s[r0:r1, 1:2]
                if eng == "a":
                    nc.scalar.activation(ob[r0:r1, :width], bt[r0:r1, :width],
                                         Ident, bias=bias, scale=1.0)
                elif eng == "s":
                    nc.scalar.activation(ob[r0:r1, 0:512], bt[r0:r1, 0:512],
                                         Ident, bias=bias, scale=1.0)
                    nc.vector.tensor_scalar(ob[r0:r1, 512:1024],
                                            bt[r0:r1, 512:1024],
                                            bias, None, Alu.add)
                else:
                    nc.vector.tensor_scalar(ob[r0:r1, :width],
                                            bt[r0:r1, :width],
                                            bias, None, Alu.add)
                nc.sync.dma_start(
                    otb[r0:r1, 1024 * p:1024 * p + width], ob[r0:r1, :width])

            def emit_mm1(gi):
                g0, gw = gstart[gi], GSCHED[gi]
                if g0 + gw <= XBASE:
                    xtile, c0, cn = None, 0, 0
                else:
                    ci = max(c for c in xin_tiles
                             if cstart[c] <= max(g0, XBASE))
                    xtile, c0, cn = xin_tiles[ci]
                    assert g0 + gw <= c0 + cn, (gi, ci)
                xo = g0 - c0
                zp = pspool.tile([128, 1024], F32, tag="ps")
                for s0 in range(0, gw, TILE_N):
                    sn = min(TILE_N, gw - s0)
                    if g0 + s0 < XBASE:
                        # first 1024 x-cols ride along in the wab load
                        mov = was[0:28, 128 + g0 + s0:128 + g0 + s0 + sn]
                    else:
                        mov = xtile[:, xo + s0:xo + s0 + sn]
                    nc.tensor.matmul(zp[:, s0:s0 + sn], w1s, mov)
                return zp

            zp_queue = [emit_mm1(0), emit_mm1(1)]
            for gi in range(NG):
                g0, gw = gstart[gi], GSCHED[gi]
                # B copy for the completed triple (must precede new btile alloc)
                if gi % 3 == 0 and gi >= 3:
                    p = gi // 3 - 1
                    emit_bcopy(p, 0, 32 * (min(3, NG - 3 * p) - 1) + 12)
                # chunk prefetch: issue chunk k+1 when entering chunk k
                while next_chunk + 1 < len(XCHUNKS) and cstart[next_chunk] <= g0 + XBASE:
                    next_chunk += 1
                    load_chunk(next_chunk)
                if not g1_loaded and gi >= 1:
                    nc.sync.dma_start(g1s[:], g1ab[:])
                    g1_loaded = True

                zp = zp_queue.pop(0)
                ss = wpool.tile([128, 1024], F16, tag="ss")
                nc.scalar.activation(ss[:, :gw], zp[:, :gw], Sin,
                                     bias=zbias[:], scale=0.5)
                # software pipeline: mm1 runs two groups ahead of mm2
                if gi + 2 < NG:
                    zp_queue.append(emit_mm1(gi + 2))
                qq = wpool.tile([128, 1024], F16, tag="qq")
                eng = SQ_ENG[gi]
                if eng == "d":
                    nc.vector.tensor_tensor(qq[:, :gw], ss[:, :gw], ss[:, :gw],
                                            Alu.mult)
                elif eng == "A":
                    # ACT square: for the last group, ACT is idle post-sins
                    # while DVE/Pool queues are backlogged
                    nc.scalar.activation(qq[:, :gw], ss[:, :gw], Square)
                elif eng == "p":
                    nc.gpsimd.tensor_tensor(qq[:, :gw], ss[:, :gw], ss[:, :gw],
                                            Alu.mult)
                else:  # 's': split DVE/Pool; Pool gets POOL_COLS columns
                    h = max(0, gw - POOL_COLS)
                    nc.vector.tensor_tensor(qq[:, :h], ss[:, :h], ss[:, :h],
                                            Alu.mult)
                    nc.gpsimd.tensor_tensor(qq[:, h:gw], ss[:, h:gw],
                                            ss[:, h:gw], Alu.mult)

                # deferred copies go AFTER the square: the square is ready as
                # soon as the sin lands, while a copy waits its mm2 round trip
                # - emitting copies first would head-of-line block the square
                for gj in list(copy_jobs):
                    defer = DVE_DEFER if CP_ENG[gj] == "d" else ACT_DEFER
                    if gj + defer <= gi:
                        emit_copy(gj)

                opA = pspool.tile([128, 1024], F32, tag="ps")
                for s0 in range(0, gw, TILE_N):
                    sn = min(TILE_N, gw - s0)
                    nc.tensor.matmul(opA[:, s0:s0 + sn], gas,
                                     qq[:, s0:s0 + sn])
                # B: stripe 32*(gi%3) of this triple's [96, 1024] psum tile
                k = gi % 3
                if k == 0:
                    btl = psbpool.tile([96, 1024], F32, tag="bT")
                    btiles[gi // 3] = btl
                bt = btiles[gi // 3]
                r0 = 32 * k
                for s0 in range(0, gw, TILE_N):
                    sn = min(TILE_N, gw - s0)
                    nc.tensor.matmul(bt[r0:r0 + 32, s0:s0 + sn],
                                     gbs, qq[64:128, s0:s0 + sn])
                for s0 in range(gw, 1024, TILE_N):
                    # pad short stripes with duplicate data so the triple's
                    # [*, 1024] copy never reads uninitialized PSUM
                    nc.tensor.matmul(bt[r0:r0 + 32, s0:s0 + 512],
                                     gbs, qq[64:128, 0:512])

                # A copy job (deferred)
                pi = pair_of[gi]
                if pi not in och_tiles:
                    prg = STORE_PAIR[pi]
                    width = sum(GSCHED[g] for g in prg)
                    t = opool.tile([128, 2048], F16, tag="och")
                    och_tiles[pi] = (t, gstart[prg[0]], width)
                oc = g0 - och_tiles[pi][1]

                def mk(gi=gi, opA=opA, oc=oc, gw=gw, pi=pi):
                    def run():
                        t = och_tiles[pi][0]
                        if CP_ENG[gi] == "a":
                            nc.scalar.activation(t[:, oc:oc + gw], opA[:, :gw],
                                                 Ident, bias=g1as, scale=1.0)
                        elif CP_ENG[gi] == "x":
                            h = gw // 2
                            nc.scalar.activation(t[:, oc:oc + h], opA[:, :h],
                                                 Ident, bias=g1as, scale=1.0)
                            nc.vector.tensor_scalar(t[:, oc + h:oc + gw],
                                                    opA[:, h:gw], g1as, None,
                                                    Alu.add)
                        else:
                            nc.vector.tensor_scalar(t[:, oc:oc + gw],
                                                    opA[:, :gw], g1as, None,
                                                    Alu.add)
                        pending_stores[pi] -= 1
                        if pending_stores[pi] == 0:
                            emit_store(pi)
                    return run
                copy_jobs[gi] = mk()

            # flush remaining copies in group order; for a 3-group last
            # triple ending in a short group, copy the full stripes first and
            # leave only a narrow copy on the final tail
            plast = (NG - 1) // 3
            klast = (NG - 1) % 3
            split_last = klast == 2 and GSCHED[-1] <= 512
            if split_last:
                emit_bcopy(plast, 0, 64, eng=CPB_ENG[plast])
            for gj in sorted(copy_jobs):
                emit_copy(gj)
            if split_last:
                emit_bcopy(plast, 64, 76, width=GSCHED[-1], eng="d")
            else:
                emit_bcopy(plast, 0, 32 * klast + 12)

    nc.compile()
    _PROG = nc
    return nc


def _pack_inputs(x: np.ndarray, W1: np.ndarray, W2: np.ndarray):
    assert x.shape == (B, DIM), x.shape
    assert W1.shape == (DIM, 32), W1.shape
    assert W2.shape == (32, 21), W2.shape
    G = _build_G(W1, W2)                      # fp64 (32, 35)
    g1 = G.sum(axis=0)                        # (35,)

    Gfull = np.zeros((128, 140), dtype=np.float64)
    W1blk = np.zeros((K_IN, M1), dtype=np.float32)
    for t in range(SUB):
        Gfull[32 * t:32 * t + 32, 35 * t:35 * t + 35] = -2.0 * G
        W1blk[7 * t:7 * t + 7, 32 * t:32 * t + 32] = W1

    wgb = np.zeros((128, 160), dtype=np.float16)
    wgb[:, 0:128] = Gfull[:, 0:128].astype(np.float16)
    wgb[64:128, 128:140] = Gfull[64:128, 128:140].astype(np.float16)

    g1ab = np.zeros((128, 2), dtype=np.float32)
    g1A = np.concatenate([np.tile(g1, 3), g1[:23]])          # (128,)
    g1ab[:, 0] = g1A
    for k in range(3):
        g1ab[32 * k:32 * k + 12, 1] = g1[23:35]

    # xt[m][7t+f, n] = x[m*B_CORE + t*COLS + n, f]
    xr = np.ascontiguousarray(x, dtype=np.float16).reshape(NCORES, SUB, COLS, DIM)
    xtm = np.ascontiguousarray(xr.transpose(0, 1, 3, 2)).reshape(NCORES, K_IN, COLS)
    in_maps = []
    for m in range(NCORES):
        wam = np.zeros((K_IN, 1152), dtype=np.float16)
        wam[:, 0:128] = W1blk.astype(np.float16)
        wam[:, 128:1152] = xtm[m, :, 0:1024]
        in_maps.append({"xt": xtm[m], "wab": wam, "wgb": wgb, "g1ab": g1ab})
    return in_maps


def _unpack_outputs(results) -> np.ndarray:
    ot = np.stack([np.asarray(r["ot"], dtype=np.float32) for r in results])    # (8,128,COLS)
    otb = np.stack([np.asarray(r["otb"], dtype=np.float32) for r in results])  # (8,76,3072)
    out = np.zeros((NCORES, SUB, COLS, 35), dtype=np.float32)
    # A rows 0:105 -> subgroups 0..2 full
    out[:, 0:3] = ot[:, 0:105].reshape(NCORES, 3, 35, COLS).transpose(0, 1, 3, 2)
    # A rows 105:128 -> subgroup 3 outputs 0:23
    out[:, 3, :, 0:23] = ot[:, 105:128].transpose(0, 2, 1)
    # B: otb[32k+i, 1024p+c] -> subgroup 3, col gstart[3p+k]+c, output 23+i
    gstart = np.cumsum([0] + GSCHED).tolist()
    for g, gw in enumerate(GSCHED):
        p, k = g // 3, g % 3
        out[:, 3, gstart[g]:gstart[g] + gw, 23:35] = (
            otb[:, 32 * k:32 * k + 12, 1024 * p:1024 * p + gw]
            .transpose(0, 2, 1))
    return np.ascontiguousarray(out.reshape(B, 35))


def run(x, W1, W2, **spmd_kwargs):
    from concourse.bass_utils import run_bass_kernel_spmd
    nc = _get_prog()
    in_maps = _pack_inputs(np.asarray(x, dtype=np.float32),
                           np.asarray(W1, dtype=np.float32),
                           np.asarray(W2, dtype=np.float32))
    res = run_bass_kernel_spmd(nc, in_maps, core_ids=list(range(NCORES)), **spmd_kwargs)
    return _unpack_outputs(res.results), res


def kernel(x, W1, W2):
    out, _ = run(x, W1, W2)
    return out
